# revision 38
# baseline (speedup 1.0000x reference)
"""Trainium2 Bass kernel for nn_EnigmaAttention: causal RoPE attention.

Sharding: tensor-parallel over heads (16 heads / 8 cores = 2 heads per core)
for qkv projection + attention; token-parallel for the output projection.
Each core:
  1. projects q/k/v for its 2 heads in transposed layout (channels on
     partitions) directly off a host-pretransposed xT,
  2. applies RoPE (partition-swap via SBUF-SBUF DMA + sign-baked sin table),
  3. runs block-causal attention in score-transposed orientation
     (softmax without max-subtraction — scores are bounded by |q||k|/sqrt(hd);
     exp blocks are accumulated elementwise on the DVE and ONE all-ones
     stationary matmul per q-chunk turns the accumulator into broadcast
     column sums, inverted with the fast approximate DVE reciprocal),
  4. redistributes attention outputs with one AllToAll per (batch, head)
     (head-sharded -> token-sharded, 512 KiB payloads instead of a
     16 MiB AllGather), overlapping each exchange with remaining work,
  5. computes the FULL output projection for its 2x256-token slice using the
     whole Wo (kept resident in SBUF); for the last batch the h=0 half of
     every accumulation runs as its own PSUM group while the final AllToAll
     is in flight, and the halves are combined on the DVE.
Host side only slices/transposes/casts inputs and concatenates outputs.
"""
import sys

sys.path.insert(0, "/opt/trn_rl_repo")

import numpy as np
import ml_dtypes

import concourse.bass as bass
import concourse.bacc as bacc
import concourse.mybir as mybir
import concourse.tile as tile
from concourse.tile_rust import add_dep_helper
from concourse.bass_utils import run_bass_kernel_spmd

BF16 = mybir.dt.bfloat16
F32 = mybir.dt.float32
AF = mybir.ActivationFunctionType

N_CORES = 8
B, S, D = 2, 2048, 2048
H = 16
HD = D // H            # 128 head dim
HL = H // N_CORES      # 2 local heads
CH = HL * HD           # 256 local qkv channels
CHK = S // N_CORES     # 256 output-projection tokens per core per batch
TCH = 512              # token chunk
KB = 128               # k block
QB = 512               # q chunk
LAG = 3                # exp->PV software-pipeline distance
ROPE_BASE = 10000.0
# hd channels are interleaved across partitions so each RoPE rotate-half
# pair (d, d+64) sits on an adjacent partition pair (2i, 2i+1): the swap is
# then a within-quadrant stream_shuffle on the DVE instead of SBUF-SBUF DMAs.
# partition p holds channel PERM[p]; q.k dot products are invariant to it.
PERM = [(p % 2) * 64 + p // 2 for p in range(HD)]
SWAP_MASK = [j ^ 1 for j in range(32)]


def build_program(seq=S, batch=B):
    s = seq
    t = batch * seq
    ntch = t // TCH
    nkb = s // 128            # k blocks per batch row
    scale = float(HD) ** -0.5
    nd = D // 128             # 16 contraction blocks
    chk = s // N_CORES

    nc = bacc.Bacc(num_devices=N_CORES)
    # host pre-lays everything out partition-major/contiguous so every load
    # is a straight copy (strided rearranges cost ~16us of descriptor
    # generation per DMA on the issuing engine)
    xR = nc.declare_dram_parameter("xR", [ntch * 128, nd * TCH], BF16,
                                   isOutput=False)
    wqR = nc.declare_dram_parameter("wqR", [128, nd * CH], BF16,
                                    isOutput=False)
    wkR = nc.declare_dram_parameter("wkR", [128, nd * CH], BF16,
                                    isOutput=False)
    wvR = nc.declare_dram_parameter("wvR", [128, nd * CH], BF16,
                                    isOutput=False)
    woR = nc.declare_dram_parameter("woR", [128, nd * D], BF16,
                                    isOutput=False)
    cosT = nc.declare_dram_parameter("cosT", [HD, s], BF16, isOutput=False)
    sinS = nc.declare_dram_parameter("sinS", [HD, s], BF16, isOutput=False)
    outT = nc.declare_dram_parameter(
        "outT", [D, batch * chk], F32, isOutput=True
    )

    # per-(batch, head) AllToAll: row-block j of the input is this core's
    # head-h [128ch x 256tok] payload for destination rank j; after the
    # exchange row-block s holds rank s's head (2s+h) for OUR tokens.
    # Split per head so each fires as soon as that head's attention is done —
    # the first exchange absorbs inter-core skew, the last one is small.
    a2a_out = [
        [
            nc.dram_tensor(f"a2aout{b}{h}", [N_CORES * HD, chk], BF16)
            for h in range(HL)
        ]
        for b in range(batch)
    ]

    with tile.TileContext(nc) as tc:
        with (
            tc.tile_pool(name="persist", bufs=1) as persist,
            tc.tile_pool(name="dram", bufs=1, space="DRAM") as dram,
        ):
            def hb_tiles(prefix):
                return [
                    [
                        persist.tile(
                            [HD, s], BF16,
                            tag=f"{prefix}{h}{b}", name=f"{prefix}{h}{b}",
                        )
                        for b in range(batch)
                    ]
                    for h in range(HL)
                ]

            qTb = hb_tiles("qTb")
            kTb = hb_tiles("kTb")
            v_sb = [
                persist.tile([128, nkb, CH], BF16, tag=f"v{b}", name=f"v{b}")
                for b in range(batch)
            ]
            wo_sb = persist.tile([128, nd, D], BF16, tag="wo")
            a2a_in = [
                [
                    dram.tile([N_CORES * HD, chk], BF16, name=f"a2ain{b}{h}")
                    for h in range(HL)
                ]
                for b in range(batch)
            ]

            with (
                tc.tile_pool(name="att", bufs=6) as att,
                tc.tile_pool(name="accp", bufs=2) as accp,
                tc.tile_pool(name="oslp", bufs=3) as oslp,
                tc.tile_pool(name="onesp", bufs=1) as onesp,
                tc.tile_pool(name="pst", bufs=3, space="PSUM") as pst,
                tc.tile_pool(name="pot", bufs=2, space="PSUM") as pot,
                tc.tile_pool(name="psm", bufs=1, space="PSUM") as psm,
            ):
                ones_sq = onesp.tile([128, 128], BF16, tag="onesq")
                nc.vector.memset(ones_sq[:], 1.0)
                # lower-triangular keep-mask for diagonal blocks (k<=q)
                dmask = onesp.tile([128, KB], BF16, tag="dmask")
                nc.gpsimd.memset(dmask[:], 1.0)
                nc.gpsimd.affine_select(
                    out=dmask[:],
                    in_=dmask[:],
                    pattern=[[1, KB]],
                    compare_op=mybir.AluOpType.is_ge,
                    fill=0.0,
                    base=0,
                    channel_multiplier=-1,
                )

                def attention_piece(b, h):
                    for qc in range(s // QB):
                        ot = pot.tile([128, QB], F32, tag="ot", name="ot")
                        # f32 running sum of the exp blocks (DVE) replaces the
                        # per-block ones-matmul: one tensor-engine column sum
                        # per q chunk instead of nkj of them.
                        eacc = accp.tile([128, QB], F32, tag="eacc",
                                         name="eacc")
                        ebf = accp.tile([128, QB], BF16, tag="ebf", name="ebf")
                        nkj = (qc + 1) * (QB // KB)
                        nfull = qc * (QB // KB)  # full-width k blocks
                        pend = []

                        def flush(one):
                            kj, e_ap, width, qoff = one
                            nc.tensor.matmul(
                                ot[:, qoff:],
                                lhsT=v_sb[b][:, kj, h * HD : (h + 1) * HD],
                                rhs=e_ap[:, :width],
                                start=(kj == 0),
                                stop=(kj == nkj - 1),
                            )

                        def qk(st_slot, kj, qoff, width):
                            nc.tensor.matmul(
                                st_slot[:, :width],
                                lhsT=kTb[h][b][:, kj * KB : (kj + 1) * KB],
                                rhs=qTb[h][b][
                                    :, qc * QB + qoff : (qc + 1) * QB
                                ],
                                start=True,
                                stop=True,
                            )

                        def post_block(kj, e_ap, width, qoff):
                            if kj == 0:
                                nc.vector.tensor_copy(eacc[:], e_ap[:, :QB])
                            else:
                                nc.vector.tensor_add(
                                    eacc[:, qoff:], eacc[:, qoff:],
                                    e_ap[:, :width],
                                )
                            pend.append((kj, e_ap, width, qoff))
                            if len(pend) > LAG:
                                flush(pend.pop(0))

                        for kj in range(nkj):
                            qoff = max(0, kj * KB - qc * QB)
                            width = QB - qoff
                            st = pst.tile([128, QB], F32, tag="st", name="st")
                            qk(st, kj, qoff, width)
                            e1 = att.tile(
                                [128, QB], BF16, tag="e", name="e", bufs=8
                            )
                            nc.scalar.activation(
                                e1[:, :width], st[:, :width], AF.Exp,
                                scale=scale,
                            )
                            if kj >= nfull:
                                # diagonal block: zero where k > q (gpsimd —
                                # vector is the busiest non-tensor engine in
                                # the attention window)
                                nc.gpsimd.tensor_mul(
                                    e1[:, :KB], e1[:, :KB], dmask[:]
                                )
                            post_block(kj, e1, width, qoff)
                        nc.vector.tensor_copy(ebf[:], eacc[:])
                        for one in pend:
                            flush(one)
                        smt = psm.tile([128, QB], F32, tag="sm", name="sm")
                        sm = smt[:]
                        nc.tensor.matmul(
                            sm, lhsT=ones_sq[:], rhs=ebf[:],
                            start=True, stop=True,
                        )
                        bcs = att.tile(
                            [128, QB], F32, tag="bcs", name="bcs", bufs=2
                        )
                        nc.vector.reciprocal_approx_fast(bcs[:], sm)
                        oslc = oslp.tile([128, QB], BF16, tag="oslc")
                        last_oslc = nc.vector.tensor_mul(
                            oslc[:], ot[:], bcs[:]
                        )
                        # stage this q-chunk's two destination-rank payloads
                        for half in range(2):
                            j = 2 * qc + half
                            nc.gpsimd.dma_start(
                                out=a2a_in[b][h][j * HD : (j + 1) * HD, :],
                                in_=oslc[:, half * chk : (half + 1) * chk],
                            )
                    nc.gpsimd.collective_compute(
                        "AllToAll",
                        mybir.AluOpType.bypass,
                        replica_groups=[list(range(N_CORES))],
                        ins=[a2a_in[b][h][:]],
                        outs=[a2a_out[b][h][:]],
                    )
                    return last_oslc

                # ------------- projections (interleaved with attention) ----
                with (
                    tc.tile_pool(name="w3", bufs=1) as w3,
                    tc.tile_pool(name="xin", bufs=3) as xin,
                    tc.tile_pool(name="trig", bufs=1) as trig,
                    tc.tile_pool(name="rope", bufs=2) as rope,
                    tc.tile_pool(name="pp", bufs=2, space="PSUM") as pp,
                ):
                    # startup-critical DMA priority: wq + first x chunk, then
                    # wk/cos, then sin/wv, then the big Wo load (not needed
                    # until out-proj).  Program order means nothing to the
                    # Tile scheduler, so priority is enforced with explicit
                    # deps between the DMAs.
                    wsbs = {}
                    for name in ("q", "k", "v"):
                        wsbs[name] = w3.tile(
                            [128, nd, CH], BF16, tag=f"w{name}", name=f"w{name}"
                        )

                    def load_w(name, wdram, after=None):
                        dma = nc.sync.dma_start(
                            out=wsbs[name][:],
                            in_=wdram.rearrange("p (a c) -> p a c", c=CH),
                        )
                        if after is not None:
                            add_dep_helper(dma.ins, after.ins, False, "dma prio")
                        return dma

                    def load_w_half(name, wdram, hf, after=None):
                        dma = nc.sync.dma_start(
                            out=wsbs[name][
                                :, hf * (nd // 2) : (hf + 1) * (nd // 2), :
                            ],
                            in_=wdram[
                                :, hf * (nd // 2) * CH : (hf + 1)
                                * (nd // 2) * CH
                            ].rearrange("p (a c) -> p a c", c=CH),
                        )
                        if after is not None:
                            add_dep_helper(dma.ins, after.ins, False, "dma prio")
                        return dma

                    def load_xc(tch, halves_tiles=None):
                        halves = []
                        dmas = []
                        for hf in range(2):
                            if halves_tiles is not None:
                                xc = halves_tiles[hf]
                            else:
                                xc = xin.tile(
                                    [128, nd // 2, TCH], BF16, tag="xc",
                                    name="xc",
                                )
                            dma = nc.sync.dma_start(
                                out=xc[:],
                                in_=xR[
                                    tch * 128 : (tch + 1) * 128,
                                    hf * (nd // 2) * TCH : (hf + 1)
                                    * (nd // 2)
                                    * TCH,
                                ].rearrange("p (a u) -> p a u", u=TCH),
                            )
                            halves.append(xc)
                            dmas.append(dma)
                        return halves, dmas

                    def load_xc0_quarter(xc, hf, pt, dep):
                        np_blk = nd // 4
                        a0 = hf * (nd // 2) + pt * np_blk
                        dma = nc.sync.dma_start(
                            out=xc[:, pt * np_blk : (pt + 1) * np_blk, :],
                            in_=xR[
                                0:128, a0 * TCH : (a0 + np_blk) * TCH
                            ].rearrange("p (a u) -> p a u", u=TCH),
                        )
                        if dep is not None:
                            add_dep_helper(dma.ins, dep.ins, False, "dma prio")
                        return dma

                    # startup chain wq0 -> xq0 -> cos -> xq1 -> sin -> wq1 ->
                    # xq2 -> xq3 -> wk -> wv: quarter-granularity chunk-0 x
                    # loads let the first contraction-block matmuls start
                    # after ~1 MiB of DMA, and the small trig tables land
                    # early so the chunk-0 rope drains never block the
                    # projection PSUM rotation.
                    wq_dma0 = load_w_half("q", wqR, 0)
                    xc0 = [
                        xin.tile([128, nd // 2, TCH], BF16, tag="xc",
                                 name="xc")
                        for _ in range(2)
                    ]
                    xq = load_xc0_quarter(xc0[0], 0, 0, wq_dma0)
                    cos_sb = trig.tile([HD, s], BF16, tag="cos")
                    cos_dma = nc.sync.dma_start(out=cos_sb[:], in_=cosT[:, :])
                    add_dep_helper(cos_dma.ins, xq.ins, False, "dma prio")
                    xq = load_xc0_quarter(xc0[0], 0, 1, cos_dma)
                    sin_sb = trig.tile([HD, s], BF16, tag="sin")
                    sin_dma = nc.sync.dma_start(out=sin_sb[:], in_=sinS[:, :])
                    add_dep_helper(sin_dma.ins, xq.ins, False, "dma prio")
                    wq_dma1 = load_w_half("q", wqR, 1, after=sin_dma)
                    xq = load_xc0_quarter(xc0[1], 1, 0, wq_dma1)
                    last_xq0 = load_xc0_quarter(xc0[1], 1, 1, xq)
                    wk_dma = load_w("k", wkR, after=last_xq0)
                    wv_dma = load_w("v", wvR, after=wk_dma)

                    def rope_drain(ps, dest, h, b, sc):
                        zc = rope.tile([128, TCH], F32, tag="zc")
                        nc.vector.tensor_mul(
                            zc[:], ps[:], cos_sb[:, sc : sc + TCH]
                        )
                        zs = rope.tile([128, TCH], F32, tag="zs")
                        nc.vector.stream_shuffle(zs[:], ps[:], SWAP_MASK)
                        nc.vector.tensor_mul(
                            zs[:], zs[:], sin_sb[:, sc : sc + TCH]
                        )
                        nc.vector.tensor_add(
                            dest[h][b][:, sc : sc + TCH], zc[:], zs[:]
                        )

                    def proj_chunk(tch, pre=None):
                        b = (tch * TCH) // s
                        sc = tch * TCH - b * s
                        xcs = pre if pre is not None else load_xc(tch)[0]
                        if tch == 0:
                            # chunk 0 is DMA-paced: interleave the two heads'
                            # accumulations per contraction block so matmul
                            # availability tracks the quarter-granularity x
                            # arrivals instead of stalling on the full chunk.
                            for wname, dest in (("q", qTb), ("k", kTb)):
                                pss = [
                                    pp.tile([128, TCH], F32, tag="mm",
                                            name="ps")
                                    for _ in range(HL)
                                ]
                                for dblk in range(nd):
                                    for h in range(HL):
                                        nc.tensor.matmul(
                                            pss[h][:],
                                            lhsT=wsbs[wname][
                                                :, dblk,
                                                h * HD : (h + 1) * HD,
                                            ],
                                            rhs=xcs[dblk // 8][
                                                :, dblk % 8, :
                                            ],
                                            start=(dblk == 0),
                                            stop=(dblk == nd - 1),
                                        )
                                for h in range(HL):
                                    rope_drain(pss[h], dest, h, b, sc)
                        else:
                            for h in range(HL):
                                for wname, dest in (("q", qTb), ("k", kTb)):
                                    ps = pp.tile(
                                        [128, TCH], F32, tag="mm", name="ps"
                                    )
                                    for dblk in range(nd):
                                        nc.tensor.matmul(
                                            ps[:],
                                            lhsT=wsbs[wname][
                                                :, dblk,
                                                h * HD : (h + 1) * HD,
                                            ],
                                            rhs=xcs[dblk // 8][
                                                :, dblk % 8, :
                                            ],
                                            start=(dblk == 0),
                                            stop=(dblk == nd - 1),
                                        )
                                    rope_drain(ps, dest, h, b, sc)
                        for tp in range(TCH // 128):
                            vps = pp.tile([128, CH], F32, tag="mm", name="vps")
                            for dblk in range(nd):
                                nc.tensor.matmul(
                                    vps[:],
                                    lhsT=xcs[dblk // 8][
                                        :, dblk % 8, tp * 128 : (tp + 1) * 128
                                    ],
                                    rhs=wsbs["v"][:, dblk, :],
                                    start=(dblk == 0),
                                    stop=(dblk == nd - 1),
                                )
                            nc.scalar.activation(
                                v_sb[b][:, sc // 128 + tp, :], vps[:], AF.Copy
                            )

                    # batch-0 projection, then batch-0 attention overlapped
                    # with batch-1 projection, then batch-1 attention
                    proj_chunk(0, pre=xc0)
                    per_b = (s // TCH) if batch > 1 else ntch
                    for tch in range(1, per_b):
                        proj_chunk(tch)
                    if batch > 1:
                        for h in range(HL):
                            attention_piece(0, h)
                        last_xdma = None
                        for tch in range(per_b, ntch):
                            xcs, xdmas = load_xc(tch)
                            proj_chunk(tch, pre=xcs)
                            last_xdma = xdmas[1]
                        # Wo is 8 MiB and not needed until out-proj. Issue it
                        # from the SAME engine (sync) as the x-chunk loads so
                        # it lands on the same hardware DMA queue BEHIND them:
                        # queue order, not dep order, is what actually
                        # serializes the transfers (a dep on the DMA
                        # instruction only orders the descriptor push, and a
                        # separate queue then round-robins the bus 50/50,
                        # starving the startup-critical x stream).
                        wo_dma = nc.sync.dma_start(
                            out=wo_sb[:],
                            in_=woR.rearrange("p (a c) -> p a c", c=D),
                        )
                        add_dep_helper(
                            wo_dma.ins, last_xdma.ins, False, "dma prio"
                        )
                        for h in range(HL):
                            attention_piece(1, h)
                    else:
                        for h in range(HL):
                            attention_piece(0, h)
                        nc.sync.dma_start(
                            out=wo_sb[:],
                            in_=woR.rearrange("p (a c) -> p a c", c=D),
                        )

                # ---------------- output projection (token-sharded) --------
                with (
                    tc.tile_pool(name="ocin", bufs=2) as ocin,
                    tc.tile_pool(name="osout", bufs=3) as osout,
                    tc.tile_pool(name="pout", bufs=2, space="PSUM") as pout,
                ):
                    # h=0 channel blocks first: their AllToAll lands one
                    # attention piece earlier than h=1's, so for the last
                    # batch the first half of each accumulation can run
                    # while the final exchange is still in flight.
                    gorder = [g for g in range(nd) if g % HL == 0] + [
                        g for g in range(nd) if g % HL != 0
                    ]
                    evens = gorder[: nd // 2]
                    odds = gorder[nd // 2 :]

                    def oproj_mm(ps, ob2, ob, occ, glist, fresh, close):
                        for gi, g in enumerate(glist):
                            nc.tensor.matmul(
                                ps[:, ob2, :],
                                lhsT=wo_sb[:, g, ob * 128 : (ob + 1) * 128],
                                rhs=occ[:, g // HL, g % HL, :],
                                start=(fresh and gi == 0),
                                stop=(close and gi == len(glist) - 1),
                            )

                    for b in range(batch):
                        occ = ocin.tile(
                            [128, N_CORES, HL, chk], BF16, tag="occ",
                            name="occ",
                        )
                        for h in range(HL):
                            nc.sync.dma_start(
                                out=occ[:, :, h, :],
                                in_=a2a_out[b][h].rearrange(
                                    "(r p) t -> p r t", p=128
                                ),
                            )
                        # two 128-row output blocks share each PSUM bank
                        # (a PSUM pool buf is a whole bank): 2 bufs x 2
                        # blocks in flight, half the drain/store count.
                        if b == 0:
                            for obp in range(D // 256):
                                ps = pout.tile(
                                    [128, 2, chk], F32, tag="po", name="po"
                                )
                                for ob2 in range(2):
                                    oproj_mm(
                                        ps, ob2, obp * 2 + ob2, occ, gorder,
                                        True, True,
                                    )
                                osb = osout.tile(
                                    [128, 2, chk], F32, tag="osb", name="osb"
                                )
                                # vector drain: the scalar queue is still
                                # full of batch-1 attention exps
                                nc.vector.tensor_copy(osb[:], ps[:])
                                nc.sync.dma_start(
                                    out=outT[
                                        obp * 256 : (obp + 1) * 256, 0:chk
                                    ].rearrange("(o p) t -> p o t", p=128),
                                    in_=osb[:],
                                )
                        else:
                            # batch 1: the h=0 halves are complete PSUM
                            # groups drained to SBUF partials while the h=1
                            # AllToAll is still in flight; the h=1 halves +
                            # a DVE combine run after it lands. This keeps
                            # the tensor engine busy through the last
                            # exchange instead of idling ~17us.
                            partials = []
                            for obp in range(D // 256):
                                ps = pout.tile(
                                    [128, 2, chk], F32, tag="po", name="po"
                                )
                                for ob2 in range(2):
                                    oproj_mm(
                                        ps, ob2, obp * 2 + ob2, occ, evens,
                                        True, True,
                                    )
                                part = osout.tile(
                                    [128, 2, chk], F32, tag="part",
                                    name="part", bufs=8,
                                )
                                nc.vector.tensor_copy(part[:], ps[:])
                                partials.append(part)
                            for obp in range(D // 256):
                                ps = pout.tile(
                                    [128, 2, chk], F32, tag="po", name="po"
                                )
                                for ob2 in range(2):
                                    oproj_mm(
                                        ps, ob2, obp * 2 + ob2, occ, odds,
                                        True, True,
                                    )
                                osb = osout.tile(
                                    [128, 2, chk], F32, tag="osb", name="osb"
                                )
                                nc.vector.tensor_add(
                                    osb[:], ps[:], partials[obp][:]
                                )
                                nc.sync.dma_start(
                                    out=outT[
                                        obp * 256 : (obp + 1) * 256,
                                        chk : 2 * chk,
                                    ].rearrange("(o p) t -> p o t", p=128),
                                    in_=osb[:],
                                )
    nc.finalize()
    return nc


def host_inputs(x, Wq, Wk, Wv, Wo, seq=S, batch=B):
    """Slice/transpose/cast the full inputs into per-core input maps."""
    bf = ml_dtypes.bfloat16
    t = batch * seq
    nd = D // 128
    ntch = t // TCH
    x = np.asarray(x, dtype=np.float32)
    # xR[c, p, a, i] = x-token (c*TCH+i), channel (a*128+p)
    xT = x.reshape(t, D).T                                   # [D, t]
    xR = np.ascontiguousarray(
        xT.reshape(nd, 128, ntch, TCH).transpose(2, 1, 0, 3)
    ).astype(bf).reshape(ntch * 128, nd * TCH)

    def wlayout(w2d, cols):  # [D, cols] -> [128, nd*cols] partition-major
        return np.ascontiguousarray(
            w2d.reshape(nd, 128, cols).transpose(1, 0, 2)
        ).astype(bf).reshape(128, nd * cols)

    inv_freq = 1.0 / (
        ROPE_BASE ** (np.arange(0, HD, 2, dtype=np.float32) / HD)
    )
    pos = np.arange(seq, dtype=np.float32)
    freqs = np.einsum("i,j->ij", pos, inv_freq)
    emb = np.concatenate([freqs, freqs], axis=-1)            # [s, HD]
    perm = np.asarray(PERM)
    cosT_np = np.ascontiguousarray(np.cos(emb).T[perm]).astype(bf)
    sinT = np.sin(emb).T.astype(np.float32)
    sinS_np = np.ascontiguousarray(
        np.concatenate([-sinT[: HD // 2], sinT[HD // 2 :]], axis=0)[perm]
    ).astype(bf)

    def permute_heads(w_t):  # [D, CH]: interleave each head's hd columns
        w4 = w_t.reshape(D, HL, HD)[:, :, perm]
        return w4.reshape(D, CH)

    woR_np = wlayout(np.asarray(Wo).T, D)
    in_maps = []
    for c in range(N_CORES):
        sl = slice(c * CH, (c + 1) * CH)
        in_maps.append(
            {
                "xR": xR,
                "wqR": wlayout(permute_heads(np.asarray(Wq)[sl].T), CH),
                "wkR": wlayout(permute_heads(np.asarray(Wk)[sl].T), CH),
                "wvR": wlayout(np.asarray(Wv)[sl].T, CH),
                "woR": woR_np,
                "cosT": cosT_np,
                "sinS": sinS_np,
            }
        )
    return in_maps


def assemble_output(results, seq=S, batch=B):
    """results[c]['outT'] is [D, batch*chk]: core c's token slice per batch."""
    chk = seq // N_CORES
    out = np.empty((batch, seq, D), dtype=np.float32)
    for c in range(N_CORES):
        o = results[c]["outT"]
        for b in range(batch):
            out[b, c * chk : (c + 1) * chk, :] = o[
                :, b * chk : (b + 1) * chk
            ].T
    return out


_PROGRAM = None


def kernel(x, Wq, Wk, Wv, Wo):
    global _PROGRAM
    if _PROGRAM is None:
        _PROGRAM = build_program()
    in_maps = host_inputs(x, Wq, Wk, Wv, Wo)
    res = run_bass_kernel_spmd(_PROGRAM, in_maps, list(range(N_CORES)))
    return assemble_output(res.results)


if __name__ == "__main__":
    xs = np.random.randn(B, S, D).astype(np.float32)
    ws = [
        (np.random.randn(D, D) * D**-0.5).astype(np.float32) for _ in range(4)
    ]
    out = kernel(xs, *ws)
    print(out.shape, out.dtype)

